# revision 13
# baseline (speedup 1.0000x reference)
"""
Trainium2 Bass kernel for nn_Block_16853451670038 (moe_routing).

Strategy: data-parallel over (batch, token-half) -> 8 cores, no collectives.
Each core: own 1024 tokens permuted first, K/V over all 2048 ctx tokens.

v2: top-2 routed (gathered) MoE + transpose-free gating/QKV.
 - gating from host-provided xT hi/lo: logits = (simT.x - m*S)*rstd*ninv - sg
 - QKV: ae = xT_hi * (rw*rstd) with rank-1 (-wsum x rw*m*rstd) correction
 - gating-2 = simT.xT(3-pass) + simT.attn_outT(bf16) with LN2-mean correction
 - MoE: per-expert token gather (CAP=352) via on-device rank/slot lists,
   indirect DMA gather -> PE transpose -> w1/gelu/w2 -> rw-scaled scatter
   to A/B DRAM tables -> out = hs + A + B.
"""

import sys

for _p in ("/opt/trn_rl_repo",):
    if _p not in sys.path:
        sys.path.insert(0, _p)

import numpy as np
import ml_dtypes
from contextlib import ExitStack

import concourse.bass as bass
import concourse.tile as tile
from concourse import mybir, bacc
from concourse import bass_utils
from concourse.masks import make_identity

BF16 = ml_dtypes.bfloat16
F32 = mybir.dt.float32
BF = mybir.dt.bfloat16
I32 = mybir.dt.int32
AL = mybir.AluOpType
AF = mybir.ActivationFunctionType
AX = mybir.AxisListType

B, T, C, H = 4, 2048, 1024, 128
E = 8
TO = T // 2          # own tokens per core = 1024
N_CORES = 8
CT = C // 128        # 8 channel chunks
KT = T // 128        # 16 key tiles
MT = TO // 128       # 8 own token tiles
NCH = T // 512       # 4 ctx chunks
MCH = TO // 512      # 2 own chunks
CAP = 320            # per-expert token capacity (max measured load 297)
SUB = 3              # slot subtiles per expert: 128,128,96
BIG = 1e4
OOB = float(1 << 20)
EPS = 1e-5


def bc_mid(t2, G):
    """[128, n] AP -> [128, G, n] broadcast over new middle dim."""
    return bass.AP(tensor=t2.tensor, offset=t2.offset,
                   ap=[t2.ap[0], [0, G], t2.ap[1]])


def bc_last(t2, n):
    """[128, G] AP -> [128, G, n] broadcast over new last dim."""
    return bass.AP(tensor=t2.tensor, offset=t2.offset,
                   ap=[t2.ap[0], t2.ap[1], [0, n]])


def build_device_kernel(ctx: ExitStack, tc: tile.TileContext, io: dict):
    nc = tc.nc
    RT = float(np.sqrt(C))

    const = ctx.enter_context(tc.tile_pool(name="const", bufs=1))
    small = ctx.enter_context(tc.tile_pool(name="small", bufs=2))

    eps_t = const.tile([128, 1], F32)
    nc.vector.memset(eps_t, EPS)
    ident128b = const.tile([128, 128], BF)
    make_identity(nc, ident128b)
    ident8f = const.tile([8, 8], F32)
    make_identity(nc, ident8f)
    ident2f = const.tile([2, 2], F32)
    make_identity(nc, ident2f)
    ones128b = const.tile([128, 128], BF)
    nc.vector.memset(ones128b, 1.0)
    onescol_b = const.tile([128, 1], BF)
    nc.vector.memset(onescol_b, 1.0)
    onesrow_f = const.tile([1, 128], F32)
    nc.vector.memset(onesrow_f, 1.0)

    iota_row_i = const.tile([128, CAP], I32)
    nc.gpsimd.iota(iota_row_i, pattern=[[1, CAP]], base=0, channel_multiplier=0)
    iota352f = const.tile([128, CAP], F32)
    nc.vector.tensor_copy(out=iota352f, in_=iota_row_i)
    iota_col_i = const.tile([128, 1], I32)
    nc.gpsimd.iota(iota_col_i, pattern=[[0, 1]], base=0, channel_multiplier=1)
    iota_colf = const.tile([128, 1], F32)
    nc.vector.tensor_copy(out=iota_colf, in_=iota_col_i)
    iotasub_i = const.tile([128, SUB], I32)  # p + 128*s
    nc.gpsimd.iota(iotasub_i, pattern=[[128, SUB]], base=0, channel_multiplier=1)
    iotasubf = const.tile([128, SUB], F32)
    nc.vector.tensor_copy(out=iotasubf, in_=iotasub_i)
    # strictly-lower triangular ones: TRI[p, i] = 1 if i > p
    tri128b = const.tile([128, 128], BF)
    nc.vector.tensor_tensor(out=tri128b, in0=iota352f[:, 0:128],
                            in1=iota_colf.to_broadcast([128, 128]), op=AL.is_gt)

    with tc.tile_pool(name="ps_cn", bufs=2, space="PSUM") as pscn:
        def bcast8(dram_row, tag):
            r = small.tile([1, 8], F32, tag=tag + "_r", name=tag + "_r")
            nc.sync.dma_start(out=r, in_=dram_row)
            ps = pscn.tile([128, 8], F32, tag="bc8", name="bc8_ps")
            nc.tensor.matmul(ps, lhsT=onesrow_f, rhs=r, start=True, stop=True)
            t = const.tile([128, 8], F32, tag=tag, name=tag)
            nc.vector.tensor_copy(out=t, in_=ps)
            return t

        sg1b = bcast8(io["sg1"], "sg1b")
        sg2b = bcast8(io["sg2"], "sg2b")
        S1b = bcast8(io["scol1"], "S1b")
        S2b = bcast8(io["scol2"], "S2b")

    def load_sim(name):
        t = const.tile([128, CT, E], BF, tag=name, name=name)
        nc.sync.dma_start(out=t, in_=io[name].rearrange("(c p) e -> p c e", p=128))
        return t

    sim1h, sim1l = load_sim("sim1_h"), load_sim("sim1_l")
    sim2h, sim2l = load_sim("sim2_h"), load_sim("sim2_l")

    # ------- persistent (ctx-long) activation tiles -------
    stat_pool = ctx.enter_context(tc.tile_pool(name="stats", bufs=1))
    sc1_ch = [stat_pool.tile([128, 4], F32, tag=f"sc1_{c}", name=f"sc1_{c}")
              for c in range(NCH)]
    msc_ch = [stat_pool.tile([128, 4], F32, tag=f"msc_{c}", name=f"msc_{c}")
              for c in range(NCH)]
    rstd_ch = [stat_pool.tile([128, 4], F32, tag=f"rst_{c}", name=f"rst_{c}")
               for c in range(NCH)]
    negm_ch = [stat_pool.tile([128, 4], F32, tag=f"ngm_{c}", name=f"ngm_{c}")
               for c in range(NCH)]
    raw2x = stat_pool.tile([8, TO], F32, tag="raw2x", name="raw2x")
    rw2_f32 = [stat_pool.tile([128, 4, 8], F32, tag=f"rw2f{c}", name=f"rw2f{c}")
               for c in range(MCH)]
    mask2_f32 = [stat_pool.tile([128, 4, 8], F32, tag=f"mk2f{c}", name=f"mk2f{c}")
                 for c in range(MCH)]

    hs_pool = ctx.enter_context(tc.tile_pool(name="hs", bufs=1))

    def ln_stats(x_ap, mv_out):
        st = small.tile([128, 2, 6], F32, tag="bnst", name="bnst")
        xg = x_ap.rearrange("p (s f) -> p s f", s=2)
        for s in range(2):
            nc.vector.bn_stats(out=st[:, s, :], in_=xg[:, s, :])
        nc.vector.bn_aggr(out=mv_out, in_=st)

    def ln_derived(mv, c_list, j, with_negm):
        # c_list = (sc1, msc, rstd, negm or None); writes column j
        sc1c, mscc, rstdc, negmc = c_list
        rstd = rstdc[:, j:j + 1] if rstdc is not None else \
            small.tile([128, 1], F32, tag="rstd_t", name="rstd_t")
        nc.scalar.activation(out=rstd, in_=mv[:, 1:2], func=AF.Sqrt,
                             bias=eps_t[:, 0:1])
        nc.vector.reciprocal(out=rstd, in_=rstd)
        r2 = small.tile([128, 1], F32, tag="r2", name="r2")
        nc.vector.tensor_tensor(out=r2, in0=rstd, in1=rstd, op=AL.mult)
        nin = small.tile([128, 1], F32, tag="nin", name="nin")
        nc.vector.tensor_scalar(out=nin, in0=r2,
                                scalar1=float(EPS / (2.0 * RT)),
                                scalar2=float(1.0 / RT),
                                op0=AL.mult, op1=AL.add)
        nc.vector.tensor_tensor(out=sc1c[:, j:j + 1], in0=rstd, in1=nin,
                                op=AL.mult)
        nc.vector.tensor_tensor(out=mscc[:, j:j + 1], in0=mv[:, 0:1],
                                in1=sc1c[:, j:j + 1], op=AL.mult)
        if with_negm:
            nc.vector.tensor_scalar_mul(out=negmc[:, j:j + 1], in0=mv[:, 0:1],
                                        scalar1=-1.0)
        return rstd

    def gating_chain(rawt, sc1c, mscc, sgb, Sb, rw_f32, mask_f32, G=4):
        g = small
        t0 = g.tile([128, G, 8], F32, tag="g_t0", name="g_t0")
        nc.vector.tensor_tensor(out=t0, in0=rawt, in1=bc_last(sc1c, 8),
                                op=AL.mult)
        t1 = g.tile([128, G, 8], F32, tag="g_t1", name="g_t1")
        nc.vector.tensor_tensor(out=t1, in0=bc_last(mscc, 8), in1=bc_mid(Sb, G),
                                op=AL.mult)
        lg = g.tile([128, G, 8], F32, tag="g_lg", name="g_lg")
        nc.vector.tensor_tensor(out=lg, in0=t0, in1=t1, op=AL.subtract)
        nc.vector.tensor_tensor(out=lg, in0=lg, in1=bc_mid(sgb, G),
                                op=AL.subtract)
        gated = g.tile([128, G, 8], F32, tag="g_gt", name="g_gt")
        nc.vector.tensor_scalar_max(out=gated, in0=lg, scalar1=0.0)
        m1 = g.tile([128, G], F32, tag="g_m1", name="g_m1")
        nc.vector.tensor_reduce(out=m1, in_=lg, axis=AX.X, op=AL.max)
        eq1 = g.tile([128, G, 8], F32, tag="g_eq", name="g_eq")
        nc.vector.tensor_tensor(out=eq1, in0=lg, in1=bc_last(m1, 8),
                                op=AL.is_equal)
        l2 = g.tile([128, G, 8], F32, tag="g_l2", name="g_l2")
        nc.vector.scalar_tensor_tensor(out=l2, in0=eq1, scalar=-BIG, in1=lg,
                                       op0=AL.mult, op1=AL.add)
        m2 = g.tile([128, G], F32, tag="g_m2", name="g_m2")
        nc.vector.tensor_reduce(out=m2, in_=l2, axis=AX.X, op=AL.max)
        topk = g.tile([128, G, 8], F32, tag="g_tk", name="g_tk")
        nc.vector.tensor_tensor(out=topk, in0=lg, in1=bc_last(m2, 8),
                                op=AL.is_ge)
        act = g.tile([128, G, 8], F32, tag="g_ac", name="g_ac")
        nc.vector.tensor_scalar(out=act, in0=gated, scalar1=0.0, scalar2=None,
                                op0=AL.is_gt)
        anyact = g.tile([128, G], F32, tag="g_an", name="g_an")
        nc.vector.tensor_reduce(out=anyact, in_=act, axis=AX.X, op=AL.max)
        dm = g.tile([128, G, 8], F32, tag="g_dm", name="g_dm")
        nc.vector.tensor_tensor(out=dm, in0=act, in1=topk, op=AL.subtract)
        nc.vector.tensor_tensor(out=dm, in0=dm, in1=bc_last(anyact, 8),
                                op=AL.mult)
        nc.vector.tensor_tensor(out=mask_f32, in0=dm, in1=topk, op=AL.add)
        t2 = g.tile([128, G, 8], F32, tag="g_t2", name="g_t2")
        nc.vector.tensor_scalar_add(out=t2, in0=gated, scalar1=BIG)
        nc.vector.tensor_tensor(out=t2, in0=t2, in1=mask_f32, op=AL.mult)
        mx = g.tile([128, G], F32, tag="g_mx", name="g_mx")
        nc.vector.tensor_reduce(out=mx, in_=t2, axis=AX.X, op=AL.max)
        nc.vector.tensor_tensor(out=t2, in0=t2, in1=bc_last(mx, 8),
                                op=AL.subtract)
        ee = g.tile([128, G, 8], F32, tag="g_ee", name="g_ee")
        nc.scalar.activation(out=ee, in_=t2, func=AF.Exp)
        ss = g.tile([128, G], F32, tag="g_ss", name="g_ss")
        nc.vector.tensor_reduce(out=ss, in_=ee, axis=AX.X, op=AL.add)
        nc.vector.reciprocal(out=ss, in_=ss)
        nc.vector.tensor_tensor(out=rw_f32, in0=ee, in1=bc_last(ss, 8),
                                op=AL.mult)

    ab_dram = io["ab_dram"]

    es_ab = ExitStack()

    es_kvq = ExitStack()
    kvq_pool = es_kvq.enter_context(tc.tile_pool(name="kvq", bufs=1))
    kT = [kvq_pool.tile([128, 512], BF, tag=f"kT{c}", name=f"kT{c}")
          for c in range(NCH)]
    vT = [kvq_pool.tile([128, 512], BF, tag=f"vT{c}", name=f"vT{c}")
          for c in range(NCH)]
    qT = [kvq_pool.tile([128, 512], BF, tag=f"qT{c}", name=f"qT{c}")
          for c in range(MCH)]
    eT_pool = es_kvq.enter_context(tc.tile_pool(name="eT", bufs=1))
    eT = [eT_pool.tile([128, TO], BF, tag=f"eT{m}", name=f"eT{m}")
          for m in range(KT)]
    v_km = eT_pool.tile([128, KT, H], BF, tag="v_km", name="v_km")

    es_xt = ExitStack()
    xthi_pool = es_xt.enter_context(tc.tile_pool(name="xthi", bufs=1))
    xthi = xthi_pool.tile([128, CT, T], BF, tag="xthi", name="xthi")

    # ======================= Phase A: LN1 + gating-1 ====================
    with tc.tile_pool(name="phA", bufs=2) as phA, \
         tc.tile_pool(name="phA_ps", bufs=2, space="PSUM") as psA:
        for c in range(NCH):
            own = c < MCH
            mvs = []
            for j in range(4):
                i = c * 4 + j
                xt = phA.tile([128, C], F32, tag="x_ctx", name="x_ctx")
                nc.scalar.dma_start(out=xt, in_=io["x"][i * 128:(i + 1) * 128, :])
                mv = small.tile([128, 2], F32, tag="mv", name="mv")
                ln_stats(xt, mv)
                mvs.append(mv)
            for j in range(4):
                ln_derived(mvs[j], (sc1_ch[c], msc_ch[c], rstd_ch[c],
                                    negm_ch[c]), j, True)

            csl = slice(c * 512, (c + 1) * 512)
            nc.sync.dma_start(
                out=xthi[:, :, csl],
                in_=io["xt_hi"].rearrange("(k p) t -> p k t", p=128)[:, :, csl])
            xtlo = phA.tile([128, CT, 512], BF, tag="xtlo", name="xtlo")
            nc.sync.dma_start(
                out=xtlo,
                in_=io["xt_lo"].rearrange("(k p) t -> p k t", p=128)[:, :, csl])

            raw_ps = psA.tile([8, 512], F32, tag="raw_ps", name="raw_ps")
            n = 0
            for (sm, xt_) in ((sim1h, xthi[:, :, csl]), (sim1l, xthi[:, :, csl]),
                              (sim1h, xtlo)):
                for k in range(CT):
                    nc.tensor.matmul(raw_ps, lhsT=sm[:, k, :], rhs=xt_[:, k, :],
                                     start=(n == 0), stop=(n == 3 * CT - 1))
                    n += 1
            raw_sb = small.tile([8, 512], F32, tag="raw_sb", name="raw_sb",
                                bufs=2)
            nc.scalar.copy(out=raw_sb, in_=raw_ps)
            rawt = small.tile([128, 4, 8], F32, tag="rawt", name="rawt", bufs=2)
            for j in range(4):
                tp = psA.tile([128, 8], F32, tag="tp", name="tp")
                nc.tensor.transpose(tp, raw_sb[:, j * 128:(j + 1) * 128], ident8f)
                nc.vector.tensor_copy(out=rawt[:, j, :], in_=tp)

            if own:
                raw2_ps = psA.tile([8, 512], F32, tag="raw2_ps", name="raw2_ps")
                n = 0
                for (sm, xt_) in ((sim2h, xthi[:, :, csl]),
                                  (sim2l, xthi[:, :, csl]), (sim2h, xtlo)):
                    for k in range(CT):
                        nc.tensor.matmul(raw2_ps, lhsT=sm[:, k, :],
                                         rhs=xt_[:, k, :],
                                         start=(n == 0), stop=(n == 3 * CT - 1))
                        n += 1
                nc.scalar.copy(out=raw2x[:, csl], in_=raw2_ps)

            rw_f = small.tile([128, 4, 8], F32, tag="rw_f", name="rw_f", bufs=2)
            mk_f = small.tile([128, 4, 8], F32, tag="mk_f", name="mk_f", bufs=2)
            gating_chain(rawt, sc1_ch[c], msc_ch[c], sg1b, S1b, rw_f, mk_f)

            alf = small.tile([128, 4, 8], F32, tag="alf", name="alf", bufs=2)
            nc.vector.tensor_tensor(out=alf, in0=rw_f,
                                    in1=bc_last(rstd_ch[c], 8), op=AL.mult)
            bef = small.tile([128, 4, 8], F32, tag="bef", name="bef", bufs=2)
            nc.vector.tensor_tensor(out=bef, in0=alf,
                                    in1=bc_last(negm_ch[c], 8), op=AL.mult)
            abr = small.tile([128, 4, 24], BF, tag="abr", name="abr", bufs=2)
            nc.vector.tensor_copy(out=abr[:, :, 0:8], in_=alf)
            nc.vector.tensor_copy(out=abr[:, :, 8:16], in_=bef)
            nc.vector.tensor_copy(out=abr[:, :, 16:24], in_=rw_f)
            for j in range(4):
                i = c * 4 + j
                tp = psA.tile([24, 128], BF, tag="tp24", name="tp24")
                nc.tensor.transpose(tp, abr[:, j, :], ident128b)
                absb = small.tile([24, 128], BF, tag="absb", name="absb", bufs=2)
                nc.vector.tensor_copy(out=absb, in_=tp)
                nc.gpsimd.dma_start(out=ab_dram[:, i * 128:(i + 1) * 128],
                                    in_=absb)

    # ======================= Phase B: QKV dense + scores ================
    mask_done = 0
    with tc.tile_pool(name="phB", bufs=2) as phB, \
         tc.tile_pool(name="wsum_p", bufs=1) as wsum_p, \
         tc.tile_pool(name="mask_p", bufs=2) as mask_p, \
         tc.tile_pool(name="zb", bufs=2) as z_pool, \
         tc.tile_pool(name="ps_kvq", bufs=1, space="PSUM") as pskvq, \
         tc.tile_pool(name="ps_s", bufs=1, space="PSUM") as pss:
        wsum = wsum_p.tile([1, E, 384], BF, tag="wsum", name="wsum")
        nc.sync.dma_start(out=wsum, in_=io["wsum"].rearrange("o (e h) -> o e h", e=E))

        def s_z_exp(m):
            mt = mask_p.tile([128, TO], BF, tag="mt", name="mt")
            nc.gpsimd.dma_start(out=mt, in_=io["maskT"][m * 128:(m + 1) * 128, :])
            for chn in range(MCH):
                nsl = slice(chn * 512, (chn + 1) * 512)
                ps_sc = pss.tile([128, 512], F32, tag="ps_sc", name="ps_sc")
                nc.tensor.matmul(
                    ps_sc, lhsT=kT[m // 4][:, (m % 4) * 128:(m % 4 + 1) * 128],
                    rhs=qT[chn], start=True, stop=True)
                z = z_pool.tile([128, 512], F32, tag="z", name="z")
                nc.vector.tensor_tensor(out=z, in0=ps_sc, in1=mt[:, nsl],
                                        op=AL.add)
                nc.scalar.activation(out=eT[m][:, nsl], in_=z, func=AF.Exp)

        for pa, chunks in ((0, (0, 1)), (1, (2, 3))):
            ps_k = {c: pskvq.tile([128, 512], F32, tag=f"ps_k{c % 2}",
                                  name=f"ps_k{c}") for c in chunks}
            ps_v = {c: pskvq.tile([128, 512], F32, tag=f"ps_v{c % 2}",
                                  name=f"ps_v{c}") for c in chunks}
            ps_q = {c: pskvq.tile([128, 512], F32, tag=f"ps_q{c % 2}",
                                  name=f"ps_q{c}") for c in chunks if c < MCH}
            alpha = {}
            for c in chunks:
                al = phB.tile([128, E, 512], BF, tag="alpha", name="alpha")
                src = ab_dram[0:8, c * 512:(c + 1) * 512]
                nc.sync.dma_start(
                    out=al, in_=bass.AP(tensor=src.tensor, offset=src.offset,
                                        ap=[[0, 128]] + src.ap))
                alpha[c] = al
            for e in range(E):
                first = (e == 0)
                wq = phB.tile([128, CT, 3 * H], BF, tag="wq", name="wq")
                nc.sync.dma_start(
                    out=wq, in_=io["wqkv"][e].rearrange("(c p) h -> p c h", p=128))
                for c in chunks:
                    csl = slice(c * 512, (c + 1) * 512)
                    ae = phB.tile([128, CT, 512], BF, tag="ae", name="ae")
                    nc.vector.tensor_tensor(out=ae, in0=xthi[:, :, csl],
                                            in1=bc_mid(alpha[c][:, e, :], CT),
                                            op=AL.mult)
                    for k in range(CT):
                        st = (first and k == 0)
                        nc.tensor.matmul(ps_k[c], lhsT=wq[:, k, H:2 * H],
                                         rhs=ae[:, k, :], start=st, stop=False)
                        nc.tensor.matmul(ps_v[c], lhsT=wq[:, k, 2 * H:3 * H],
                                         rhs=ae[:, k, :], start=st, stop=False)
                        if c in ps_q:
                            nc.tensor.matmul(ps_q[c], lhsT=wq[:, k, 0:H],
                                             rhs=ae[:, k, :], start=st,
                                             stop=False)
            # rank-1 mean corrections close each chunk's psum groups
            for c in chunks:
                bsrc = ab_dram[8:16, c * 512:(c + 1) * 512]
                beta = phB.tile([1, E, 512], BF, tag="beta", name="beta",
                                bufs=1)
                nc.sync.dma_start(out=beta, in_=bsrc)
                for e in range(E):
                    last = (e == E - 1)
                    be_r = beta[0:1, e, :]
                    nc.tensor.matmul(ps_k[c], lhsT=wsum[0:1, e, H:2 * H],
                                     rhs=be_r, start=False, stop=last)
                    nc.tensor.matmul(ps_v[c], lhsT=wsum[0:1, e, 2 * H:3 * H],
                                     rhs=be_r, start=False, stop=last)
                    if c in ps_q:
                        nc.tensor.matmul(ps_q[c], lhsT=wsum[0:1, e, 0:H],
                                         rhs=be_r, start=False, stop=last)
            for c in chunks:
                nc.scalar.copy(out=kT[c], in_=ps_k[c])
                nc.scalar.copy(out=vT[c], in_=ps_v[c])
                if c in ps_q:
                    nc.scalar.copy(out=qT[c], in_=ps_q[c])
            hi = 8 if pa == 0 else 16
            for m in range(mask_done, hi):
                s_z_exp(m)
            for kt in range(mask_done, hi):
                vtp = pss.tile([128, 128], BF, tag="vtp", name="vtp")
                nc.tensor.transpose(
                    vtp, vT[kt // 4][:, (kt % 4) * 128:(kt % 4 + 1) * 128],
                    ident128b)
                nc.scalar.copy(out=v_km[:, kt, :], in_=vtp)
            mask_done = hi

    es_xt.close()  # free xthi

    attnT = hs_pool.tile([128, TO], BF, tag="attnT", name="attnT")

    # ================== Phase C1/C2: rowsum, AV =========================
    with tc.tile_pool(name="phC12", bufs=1) as phC12, \
         tc.tile_pool(name="ps_rs", bufs=1, space="PSUM") as psrs, \
         tc.tile_pool(name="ps_tr1", bufs=4, space="PSUM") as pstr1, \
         tc.tile_pool(name="ps_av", bufs=2, space="PSUM") as psav:
        rs_ps = psrs.tile([1, TO], F32, tag="rs_ps", name="rs_ps")
        for m in range(KT):
            for chn in range(MCH):
                nsl = slice(chn * 512, (chn + 1) * 512)
                nc.tensor.matmul(rs_ps[:, nsl], lhsT=onescol_b,
                                 rhs=eT[m][:, nsl],
                                 start=(m == 0), stop=(m == KT - 1))
        rsum = small.tile([1, TO], F32, tag="rsum", name="rsum", bufs=1)
        nc.vector.reciprocal(out=rsum, in_=rs_ps)
        nc.gpsimd.dma_start(out=io["rs_dram"], in_=rsum)
        r_bc = phC12.tile([128, TO], F32, tag="r_bc", name="r_bc")
        src = io["rs_dram"]
        nc.sync.dma_start(out=r_bc,
                          in_=bass.AP(tensor=src.tensor, offset=src.offset,
                                      ap=[[0, 128]] + src.ap[1:]))
        for chn in range(MCH):
            nsl = slice(chn * 512, (chn + 1) * 512)
            ps_a = psav.tile([128, 512], F32, tag="ps_a", name="ps_a")
            for kt in range(KT):
                nc.tensor.matmul(ps_a, lhsT=v_km[:, kt, :], rhs=eT[kt][:, nsl],
                                 start=(kt == 0), stop=(kt == KT - 1))
            nc.vector.tensor_tensor(out=attnT[:, nsl], in0=ps_a,
                                    in1=r_bc[:, nsl], op=AL.mult)

    es_kvq.close()  # free kT/vT/qT/eT

    hs = [hs_pool.tile([128, C], F32, tag=f"hs{m}", name=f"hs{m}")
          for m in range(MT)]

    # ====== Phase C3: o_proj + residual + aT + gating-2 + n2 rows =======
    n2_tab = io["n2_tab"]
    with tc.tile_pool(name="phC3", bufs=1) as phC3, \
         tc.tile_pool(name="phC3s", bufs=2) as phC3s, \
         tc.tile_pool(name="ps_ao", bufs=2, space="PSUM") as psao, \
         tc.tile_pool(name="ps_r2p", bufs=1, space="PSUM") as psr2p, \
         tc.tile_pool(name="ps_tr2", bufs=2, space="PSUM") as pstr2:
        rwb = []
        for chn in range(MCH):
            t = phC3.tile([128, E, 512], BF, tag=f"rwb{chn}", name=f"rwb{chn}")
            src = ab_dram[16:24, chn * 512:(chn + 1) * 512]
            nc.sync.dma_start(
                out=t, in_=bass.AP(tensor=src.tensor, offset=src.offset,
                                   ap=[[0, 128]] + src.ap))
            rwb.append(t)
        at_e = []
        for e in range(E):
            a = phC3.tile([128, TO], BF, tag=f"at{e}", name=f"at{e}")
            for chn in range(MCH):
                nsl = slice(chn * 512, (chn + 1) * 512)
                nc.vector.tensor_tensor(out=a[:, nsl], in0=attnT[:, nsl],
                                        in1=rwb[chn][:, e, :], op=AL.mult)
            at_e.append(a)
        ow = phC3.tile([128, E, C], BF, tag="ow", name="ow")
        nc.sync.dma_start(out=ow, in_=io["ow"].rearrange("e p c -> p e c"))
        aT = phC3.tile([128, CT, TO], BF, tag="aT", name="aT")
        for m in range(MT):
            ps_ao = psao.tile([128, C], F32, tag="ps_ao", name="ps_ao")
            for e in range(E):
                for cc in range(2):
                    csl = slice(cc * 512, (cc + 1) * 512)
                    nc.tensor.matmul(ps_ao[:, csl],
                                     lhsT=at_e[e][:, m * 128:(m + 1) * 128],
                                     rhs=ow[:, e, csl],
                                     start=(e == 0), stop=(e == E - 1))
            xr = phC3s.tile([128, C], F32, tag="xr", name="xr")
            nc.scalar.dma_start(out=xr, in_=io["x"][m * 128:(m + 1) * 128, :])
            nc.vector.tensor_tensor(out=hs[m], in0=ps_ao, in1=xr, op=AL.add)
            nc.scalar.dma_start(out=io["out"][m * 128:(m + 1) * 128, :],
                                in_=hs[m])
            ao_bf = phC3s.tile([128, C], BF, tag="ao_bf", name="ao_bf")
            nc.scalar.copy(out=ao_bf, in_=ps_ao)
            for k in range(CT):
                tp = pstr2.tile([128, 128], BF, tag="atp", name="atp")
                nc.tensor.transpose(tp, ao_bf[:, k * 128:(k + 1) * 128],
                                    ident128b)
                nc.scalar.copy(out=aT[:, k, m * 128:(m + 1) * 128], in_=tp)

        # gating-2 raw + chain + n2 rows
        sc2 = [small.tile([128, 4], F32, tag=f"sc2_{c}", name=f"sc2_{c}", bufs=1)
               for c in range(MCH)]
        msc2 = [small.tile([128, 4], F32, tag=f"msc2_{c}", name=f"msc2_{c}",
                           bufs=1) for c in range(MCH)]
        for c in range(MCH):
            csl = slice(c * 512, (c + 1) * 512)
            ps_r2 = psr2p.tile([8, 512], F32, tag="ps_r2", name="ps_r2")
            for k in range(CT):
                nc.tensor.matmul(ps_r2, lhsT=sim2h[:, k, :], rhs=aT[:, k, csl],
                                 start=(k == 0), stop=(k == CT - 1))
            r2sb = small.tile([8, 512], F32, tag="r2sb", name="r2sb", bufs=2)
            nc.vector.tensor_tensor(out=r2sb, in0=ps_r2, in1=raw2x[:, csl],
                                    op=AL.add)
            rawt2 = small.tile([128, 4, 8], F32, tag="rawt2", name="rawt2",
                               bufs=2)
            for j in range(4):
                tp = psr2p.tile([128, 8], F32, tag="tp2", name="tp2")
                nc.tensor.transpose(tp, r2sb[:, j * 128:(j + 1) * 128], ident8f)
                nc.vector.tensor_copy(out=rawt2[:, j, :], in_=tp)
            for j in range(4):
                m = c * 4 + j
                mv = small.tile([128, 2], F32, tag="mv2", name="mv2")
                ln_stats(hs[m], mv)
                rstd = ln_derived(mv, (sc2[c], msc2[c], None, None), j, False)
                n2r = phC3s.tile([128, C], BF, tag="n2r", name="n2r")
                nc.vector.tensor_scalar(out=n2r, in0=hs[m], scalar1=mv[:, 0:1],
                                        scalar2=rstd, op0=AL.subtract,
                                        op1=AL.mult)
                nc.scalar.dma_start(out=n2_tab[m * 128:(m + 1) * 128, :],
                                    in_=n2r)
            gating_chain(rawt2, sc2[c], msc2[c], sg2b, S2b, rw2_f32[c],
                         mask2_f32[c])

    es_ab.close()  # free x_own, abb

    # =================== Phase D: MoE grouping lists ====================
    idn_pool = ctx.enter_context(tc.tile_pool(name="idn", bufs=1))
    idn_i32 = [idn_pool.tile([128, SUB, 1], I32, tag=f"idni{e}", name=f"idni{e}")
               for e in range(E)]
    rw_slot = [idn_pool.tile([128, SUB], F32, tag=f"rws{e}", name=f"rws{e}")
               for e in range(E)]

    with tc.tile_pool(name="phD", bufs=2) as phD, \
         tc.tile_pool(name="phD1", bufs=1) as phD1, \
         tc.tile_pool(name="ps_rk", bufs=2, space="PSUM") as psrk, \
         tc.tile_pool(name="ps_id", bufs=2, space="PSUM") as psid:
        mask_bf = [phD1.tile([128, 8], BF, tag=f"mbf{i}", name=f"mbf{i}")
                   for i in range(MT)]
        for i in range(MT):
            nc.vector.tensor_copy(out=mask_bf[i],
                                  in_=mask2_f32[i // 4][:, i % 4, :])
        cnt_ps = psrk.tile([128, 8], F32, tag="cnt_ps", name="cnt_ps")
        for i in range(MT):
            nc.tensor.matmul(cnt_ps, lhsT=ones128b, rhs=mask_bf[i],
                             start=(i == 0), stop=(i == MT - 1))
        cnts = phD1.tile([128, 8], F32, tag="cnts", name="cnts")
        nc.vector.tensor_copy(out=cnts, in_=cnt_ps)

        eq_t = [phD1.tile([128, E, CAP], F32, tag=f"eqt{i}", name=f"eqt{i}")
                for i in range(MT)]
        vals_t = [phD1.tile([128, 2, 8], F32, tag=f"valst{i}", name=f"valst{i}")
                  for i in range(MT)]
        for i in range(MT):
            mk = mask2_f32[i // 4][:, i % 4, :]
            rwv = rw2_f32[i // 4][:, i % 4, :]
            rk_ps = psrk.tile([128, 8], F32, tag="rk_ps", name="rk_ps")
            nc.tensor.matmul(rk_ps, lhsT=tri128b, rhs=mask_bf[i],
                             start=True, stop=(i == 0))
            for j in range(i):
                nc.tensor.matmul(rk_ps, lhsT=ones128b, rhs=mask_bf[j],
                                 start=False, stop=(j == i - 1))
            srel = phD.tile([128, 8], F32, tag="srel", name="srel")
            nc.vector.scalar_tensor_tensor(out=srel, in0=rk_ps, scalar=1.0,
                                           in1=mk, op0=AL.add, op1=AL.mult)
            nc.vector.tensor_scalar_add(out=srel, in0=srel, scalar1=-1.0)
            tok = phD.tile([128, 1], F32, tag="tok", name="tok")
            nc.vector.tensor_scalar_add(out=tok, in0=iota_colf,
                                        scalar1=float(128 * i))
            vals = vals_t[i]
            nc.vector.tensor_copy(out=vals[:, 0, :],
                                  in_=tok.to_broadcast([128, 8]))
            nc.vector.tensor_copy(out=vals[:, 1, :], in_=rwv)
            eq = eq_t[i]
            nc.vector.tensor_tensor(out=eq, in0=bc_last(srel, CAP),
                                    in1=bc_mid(iota352f, E), op=AL.is_equal)
        for e in range(E):
            iv_ps = psid.tile([2, CAP], F32, tag="iv_ps", name="iv_ps")
            for i in range(MT):
                nc.tensor.matmul(
                    iv_ps,
                    lhsT=bass.AP(tensor=vals_t[i].tensor,
                                 offset=vals_t[i].offset + e,
                                 ap=[vals_t[i].ap[0], [8, 2]]),
                    rhs=eq_t[i][:, e, :],
                    start=(i == 0), stop=(i == MT - 1))
            iv_sb = phD.tile([2, CAP], F32, tag="iv_sb", name="iv_sb")
            nc.vector.tensor_copy(out=iv_sb, in_=iv_ps)
            idnf = phD.tile([128, SUB, 2], F32, tag="idnf", name="idnf")
            nc.vector.memset(idnf, OOB)
            for s in range(SUB):
                ss = min(128, CAP - s * 128)
                tp = psid.tile([128, 2], F32, tag="iv_tp", name="iv_tp")
                nc.tensor.transpose(tp[0:ss, :], iv_sb[:, s * 128:s * 128 + ss],
                                    ident2f)
                nc.vector.tensor_copy(out=idnf[0:ss, s, :], in_=tp[0:ss, :])
            pad = phD.tile([128, SUB], F32, tag="pad", name="pad")
            nc.vector.tensor_tensor(
                out=pad, in0=iotasubf,
                in1=bass.AP(tensor=cnts.tensor, offset=cnts.offset + e,
                            ap=[cnts.ap[0], [0, SUB]]),
                op=AL.is_ge)
            nc.vector.scalar_tensor_tensor(out=idnf[:, :, 0:1],
                                           in0=bc_last(pad, 1), scalar=OOB,
                                           in1=idnf[:, :, 0:1],
                                           op0=AL.mult, op1=AL.add)
            nc.vector.tensor_copy(out=idn_i32[e], in_=idnf[:, :, 0:1])
            nc.vector.tensor_copy(out=rw_slot[e], in_=idnf[:, :, 1])

    # ======================= Phase E: grouped MoE =======================
    with tc.tile_pool(name="wmoe", bufs=2) as wmoe, \
         tc.tile_pool(name="moe_sc", bufs=2) as moe_sc, \
         tc.tile_pool(name="ps_h", bufs=2, space="PSUM") as psh, \
         tc.tile_pool(name="ps_o", bufs=2, space="PSUM") as pso, \
         tc.tile_pool(name="ps_t", bufs=2, space="PSUM") as pst:
        n2g_t = {}

        def emit_gather(e):
            if e >= E:
                return
            tiles = []
            for s in range(SUB):
                ss = min(128, CAP - s * 128)
                n2g = moe_sc.tile([128, C], BF, tag="n2g", name="n2g", bufs=9)
                nc.gpsimd.indirect_dma_start(
                    out=n2g[0:ss, :], out_offset=None, in_=n2_tab,
                    in_offset=bass.IndirectOffsetOnAxis(
                        ap=idn_i32[e][0:ss, s, 0:1], axis=0),
                    bounds_check=TO - 1, oob_is_err=False)
                tiles.append(n2g)
            n2g_t[e] = tiles

        emit_gather(0)
        emit_gather(1)
        for e in range(E):
            w1 = wmoe.tile([128, CT, C], BF, tag="w1", name="w1")
            nc.sync.dma_start(out=w1,
                              in_=io["w1"][e].rearrange("(k p) i -> p k i", p=128))
            w2 = wmoe.tile([128, CT, C], BF, tag="w2", name="w2")
            nc.sync.dma_start(out=w2,
                              in_=io["w2"][e].rearrange("(k p) c -> p k c", p=128))
            n2gT = moe_sc.tile([128, CT, CAP], BF, tag="n2gT", name="n2gT")
            for s in range(SUB):
                ss = min(128, CAP - s * 128)
                n2g = n2g_t[e][s]
                for k in range(CT):
                    if ss == 128 and k >= 4:
                        nc.sync.dma_start_transpose(
                            out=n2gT[:, k, s * 128:s * 128 + ss],
                            in_=n2g[:, k * 128:(k + 1) * 128])
                    else:
                        tp = pst.tile([128, 128], BF, tag="gtp", name="gtp")
                        nc.tensor.transpose(tp, n2g[:, k * 128:(k + 1) * 128],
                                            ident128b)
                        nc.scalar.copy(out=n2gT[:, k, s * 128:s * 128 + ss],
                                       in_=tp[:, 0:ss])
            del n2g_t[e]
            hg = moe_sc.tile([128, CT, CAP], BF, tag="hg", name="hg")
            for ic in range(CT):
                ps_h = psh.tile([128, CAP], F32, tag="ps_h", name="ps_h")
                for k in range(CT):
                    nc.tensor.matmul(ps_h, lhsT=w1[:, k, ic * 128:(ic + 1) * 128],
                                     rhs=n2gT[:, k, :],
                                     start=(k == 0), stop=(k == CT - 1))
                nc.scalar.activation(out=hg[:, ic, :], in_=ps_h, func=AF.Gelu)
            for s in range(SUB):
                ss = min(128, CAP - s * 128)
                ps_og = pso.tile([128, C], F32, tag="ps_og", name="ps_og")
                for ic in range(CT):
                    for cc in range(2):
                        csl = slice(cc * 512, (cc + 1) * 512)
                        nc.tensor.matmul(
                            ps_og[0:ss, csl],
                            lhsT=hg[:, ic, s * 128:s * 128 + ss],
                            rhs=w2[:, ic, csl],
                            start=(ic == 0), stop=(ic == CT - 1))
                og = moe_sc.tile([128, C], F32, tag="og", name="og", bufs=2)
                nc.vector.tensor_scalar(out=og[0:ss, :], in0=ps_og[0:ss, :],
                                        scalar1=rw_slot[e][0:ss, s:s + 1],
                                        scalar2=None, op0=AL.mult)
                nc.gpsimd.indirect_dma_start(
                    out=io["out"], out_offset=bass.IndirectOffsetOnAxis(
                        ap=idn_i32[e][0:ss, s, 0:1], axis=0),
                    in_=og[0:ss, :], in_offset=None,
                    bounds_check=TO - 1, oob_is_err=False,
                    compute_op=AL.add)
            emit_gather(e + 2)

    # ----- debug dumps -----
    if "dbg_hs" in io:
        with tc.tile_pool(name="dbg", bufs=2) as dbg:
            for m in range(MT):
                nc.scalar.dma_start(out=io["dbg_hs"][m * 128:(m + 1) * 128, :],
                                    in_=hs[m])
            for e in range(E):
                t = dbg.tile([128, SUB, 1], I32, tag="di", name="di")
                nc.vector.tensor_copy(out=t, in_=idn_i32[e])
                nc.sync.dma_start(
                    out=io["dbg_idn"][e * 128:(e + 1) * 128, :],
                    in_=t.rearrange("p s v -> p (s v)"))
            for i in range(MT):
                t2 = dbg.tile([128, C], BF, tag="dn", name="dn")
                nc.sync.dma_start(out=t2, in_=n2_tab[i * 128:(i + 1) * 128, :])
                nc.scalar.dma_start(out=io["dbg_n2"][i * 128:(i + 1) * 128, :],
                                    in_=t2)


# ============================= host side ====================================

_CACHE = {}


DEBUG = False


def _build():
    if "nc" in _CACHE:
        return _CACHE["nc"]
    nc = bacc.Bacc("TRN2", target_bir_lowering=False, debug=False,
                   num_devices=N_CORES)
    io = {}
    io["x"] = nc.dram_tensor("x", [T, C], F32, kind="ExternalInput").ap()
    io["xt_hi"] = nc.dram_tensor("xt_hi", [C, T], BF, kind="ExternalInput").ap()
    io["xt_lo"] = nc.dram_tensor("xt_lo", [C, T], BF, kind="ExternalInput").ap()
    io["maskT"] = nc.dram_tensor("maskT", [T, TO], BF, kind="ExternalInput").ap()
    for nm in ("sim1_h", "sim1_l", "sim2_h", "sim2_l"):
        io[nm] = nc.dram_tensor(nm, [C, E], BF, kind="ExternalInput").ap()
    for nm in ("sg1", "sg2", "scol1", "scol2"):
        io[nm] = nc.dram_tensor(nm, [1, E], F32, kind="ExternalInput").ap()
    io["wqkv"] = nc.dram_tensor("wqkv", [E, C, 3 * H], BF, kind="ExternalInput").ap()
    io["wsum"] = nc.dram_tensor("wsum", [1, E * 3 * H], BF, kind="ExternalInput").ap()
    io["ow"] = nc.dram_tensor("ow", [E, H, C], BF, kind="ExternalInput").ap()
    io["w1"] = nc.dram_tensor("w1", [E, C, C], BF, kind="ExternalInput").ap()
    io["w2"] = nc.dram_tensor("w2", [E, C, C], BF, kind="ExternalInput").ap()
    io["out"] = nc.dram_tensor("out", [TO, C], F32, kind="ExternalOutput").ap()
    io["n2_tab"] = nc.dram_tensor("n2_tab", [TO, C], BF, kind="Internal").ap()
    io["ab_dram"] = nc.dram_tensor("ab_dram", [24, T], BF, kind="Internal").ap()
    io["rs_dram"] = nc.dram_tensor("rs_dram", [1, TO], F32, kind="Internal").ap()
    if DEBUG:
        io["dbg_hs"] = nc.dram_tensor("dbg_hs", [TO, C], F32, kind="ExternalOutput").ap()
        io["dbg_idn"] = nc.dram_tensor("dbg_idn", [E * 128, SUB], I32, kind="ExternalOutput").ap()
        io["dbg_n2"] = nc.dram_tensor("dbg_n2", [TO, C], BF, kind="ExternalOutput").ap()

    with tile.TileContext(nc) as tc:
        with ExitStack() as ctx:
            build_device_kernel(ctx, tc, io)
    nc.compile()
    _CACHE["nc"] = nc
    return nc


def _host_prep(inputs):
    x = np.asarray(inputs["x"], np.float32)

    def tobf(a):
        return np.ascontiguousarray(np.asarray(a, np.float32).astype(BF16))

    def normalize_cols(s):
        n = np.linalg.norm(s, axis=0, keepdims=True)
        return s / np.maximum(n, 1e-12)

    sim1 = normalize_cols(np.asarray(inputs["smha_sim"], np.float32))
    sim2 = normalize_cols(np.asarray(inputs["moe_sim"], np.float32))
    sim1_h = tobf(sim1)
    sim1_l = tobf(sim1 - sim1_h.astype(np.float32))
    sim2_h = tobf(sim2)
    sim2_l = tobf(sim2 - sim2_h.astype(np.float32))
    sg1 = (1.0 / (1.0 + np.exp(-np.asarray(inputs["smha_gates"], np.float32)))).reshape(1, E)
    sg2 = (1.0 / (1.0 + np.exp(-np.asarray(inputs["moe_gates"], np.float32)))).reshape(1, E)
    scol1 = np.sum(sim1.astype(np.float64), axis=0).astype(np.float32).reshape(1, E)
    scol2 = np.sum(sim2.astype(np.float64), axis=0).astype(np.float32).reshape(1, E)

    qs = np.asarray(inputs["q_proj"], np.float32) * (1.0 / np.sqrt(H))
    wqkv_f = np.concatenate(
        [qs, np.asarray(inputs["k_proj"], np.float32),
         np.asarray(inputs["v_proj"], np.float32)], axis=2)
    wqkv = tobf(wqkv_f)
    wsum = tobf(np.sum(wqkv.astype(np.float64), axis=1).astype(np.float32).reshape(1, E * 3 * H))
    ow = tobf(inputs["o_proj"])
    w1 = tobf(inputs["w1"])
    w2 = tobf(inputs["w2"])

    NEG = -3e4
    own_blk = np.where(np.arange(TO)[:, None] <= np.arange(TO)[None, :], 0.0,
                       NEG).astype(np.float32)
    mask_even = tobf(np.concatenate(
        [own_blk, np.full((TO, TO), NEG, np.float32)], axis=0))
    mask_odd = tobf(np.concatenate(
        [own_blk, np.zeros((TO, TO), np.float32)], axis=0))

    in_maps = []
    for cidx in range(N_CORES):
        b, h = cidx // 2, cidx % 2
        if h == 0:
            xc = x[b]
        else:
            xc = np.concatenate([x[b, TO:], x[b, :TO]], axis=0)
        xt = np.ascontiguousarray(xc.T)
        xt_hi = tobf(xt)
        xt_lo = tobf(xt - xt_hi.astype(np.float32))
        m = {
            "x": np.ascontiguousarray(xc),
            "xt_hi": xt_hi, "xt_lo": xt_lo,
            "maskT": mask_even if h == 0 else mask_odd,
            "sim1_h": sim1_h, "sim1_l": sim1_l,
            "sim2_h": sim2_h, "sim2_l": sim2_l,
            "sg1": sg1, "sg2": sg2, "scol1": scol1, "scol2": scol2,
            "wqkv": wqkv, "wsum": wsum, "ow": ow,
            "w1": w1, "w2": w2,
        }
        in_maps.append(m)
    return in_maps


def kernel(**inputs):
    nc = _build()
    in_maps = _host_prep(inputs)
    res = bass_utils.run_bass_kernel_spmd(nc, in_maps, core_ids=list(range(N_CORES)))
    out = np.empty((B, T, C), np.float32)
    for c in range(N_CORES):
        b, h = c // 2, c % 2
        out[b, h * TO:(h + 1) * TO, :] = res.results[c]["out"]
    return out


if __name__ == "__main__":
    import reference as R
    inp = {k: np.asarray(v) for k, v in R.setup_inputs().items()}
    got = kernel(**inp)
    import jax.numpy as jnp
    exp = np.asarray(R.reference(**{k: jnp.asarray(v) for k, v in inp.items()}))
    d = np.abs(got - exp)
    print("absmax rel:", d.max() / np.abs(exp).max(),
          "L2 rel:", np.linalg.norm(d) / np.linalg.norm(exp))
